# revision 1
# baseline (speedup 1.0000x reference)
"""Trainium2 Bass kernel for nn_ChamferLossSelf (B=4, N=4096, D=3).

Math (per batch b):
  P[i,j] = ||g_i - p_j||^2   (cross);  P1 = ||g_i - g_j||^2, P2 = ||p_i - p_j||^2
  loss = sum_j min_i P + sum_i min_j P + sum_r (sort(minsP1) - sort(minsP2))^2
  where minsPk = per-point NN distance (diag excluded).

Sharding: batch b -> cores (2b, 2b+1).  Core 2b:  rows=gts, cross cols=preds,
self=gts.  Core 2b+1: rows=preds, cross cols=gts, self=preds.  Each core
computes its cross-matrix row-mins (summed -> partial) and its self-matrix
NN-distance vector (sorted on-device via a normalized-bitonic network).  An
8-core AllGather shares (sorted vector, partial, sum-of-squares); every core
then computes the 4 final scalars identically; the host reads core 0.

Distances via one K=21 bf16 matmul per tile: each f32 coordinate is split
exactly into 3 bf16 terms (h+m+l); product classes hh,hm,mh,hl,lh,mm plus a
3-way bf16 split of ||y||^2 against ones-rows give fp32-level accuracy
(~2e-7) at 1 PE cycle/row.  ||x||^2 is added after the row-min in f32.
"""

import numpy as np

import concourse.bass as bass
import concourse.bacc as bacc
import concourse.bass_isa as bass_isa
import concourse.tile as tile
from concourse import mybir
from concourse.bass_utils import run_bass_kernel_spmd

F32 = mybir.dt.float32
BF16 = mybir.dt.bfloat16
AX = mybir.AxisListType
OP = mybir.AluOpType
ACTF = mybir.ActivationFunctionType

N = 4096
NP, NT = 128, 32  # sort grid [partitions, free]; s = p*NT + t
N_CORES = 8
DIAG_BIG = 1.0e6
ALPHA = 1.0

# ---------------------------------------------------------------------------
# Sort network codegen: normalized bitonic (flip merges), all-ascending.
# Grid [128, 32], sort index s = p*32 + t.  See sortnet.py for the validated
# reference implementation of exactly this op stream.
# ---------------------------------------------------------------------------


def _plain_sel(axis_len, k):
    return [[2 * k, axis_len // (2 * k)], [1, k]]


def _sort_stages():
    ops = []
    layout = "G"

    def need(lay):
        nonlocal layout
        if layout != lay:
            ops.append(("transpose", "G2GT" if lay == "GT" else "GT2G"))
            layout = lay

    for m in range(1, 13):
        size = 1 << m
        if size <= NT:
            need("G")
            half = size // 2
            nblk = NT // size
            lo = ([[size, nblk], [1, half]], 0)
            hi = ([[size, nblk], [1, half]], half)
            lo_mir = ([[size, nblk], [-1, half]], size - 1)
            hi_mir = ([[size, nblk], [-1, half]], half - 1)
            ops.append(("stage", "G", [
                (lo, lo, lo_mir, "min", False),
                (hi, hi, hi_mir, "max", False),
            ]))
        else:
            need("GT")
            ops.append(("shuffle_rev",))
            sp = size // NT
            half = sp // 2
            nblk = NP // sp
            lo = ([[sp, nblk], [1, half]], 0)
            hi = ([[sp, nblk], [1, half]], half)
            lo_mir = ([[sp, nblk], [-1, half]], sp - 1)
            hi_mir = ([[sp, nblk], [-1, half]], half - 1)
            ops.append(("stage", "GT", [
                (lo, lo, lo_mir, "min", True),
                (hi, hi, hi_mir, "max", True),
            ]))
        k = size // 4
        while k >= 1:
            if k >= NT:
                need("GT")
                kp = k // NT
                sel = _plain_sel(NP, kp)
                ops.append(("stage", "GT", [
                    ((sel, 0), (sel, 0), (sel, kp), "min", False),
                    ((sel, kp), (sel, 0), (sel, kp), "max", False),
                ]))
            else:
                need("G")
                sel = _plain_sel(NT, k)
                ops.append(("stage", "G", [
                    ((sel, 0), (sel, 0), (sel, k), "min", False),
                    ((sel, k), (sel, 0), (sel, k), "max", False),
                ]))
            k //= 2
    need("G")
    return ops


def _sel_ap(t, sel, rowsz, nparts):
    pairs, off = sel
    return bass.AP(t.tensor, t.offset + off, [[rowsz, nparts]] + [list(p) for p in pairs])


def _emit_sort(nc, pool, psp, M, identf, sfx=""):
    """Sort the 4096 f32 values of grid M [128, 32] ascending (s = p*32+t).
    Returns the sorted G-layout grid tile."""
    G = [pool.tile([NP, NT], F32, name=f"srt_g0{sfx}"), pool.tile([NP, NT], F32, name=f"srt_g1{sfx}")]
    T = [pool.tile([NT, NP], F32, name=f"srt_t0{sfx}"), pool.tile([NT, NP], F32, name=f"srt_t1{sfx}")]
    R = pool.tile([NT, NP], F32, name=f"srt_rev{sfx}")
    nc.vector.tensor_copy(G[0][:], M[:])
    gi, ti = 0, 0
    lay = "G"
    for op in _sort_stages():
        if op[0] == "transpose":
            if op[1] == "G2GT":
                ps = psp.tile([NT, NP], F32, tag="tp", bufs=2)
                nc.tensor.transpose(ps[:], G[gi][:], identf[:])
                nc.scalar.copy(T[ti][:], ps[:])
                lay = "GT"
            else:
                ps = psp.tile([NP, NT], F32, tag="tp", bufs=2)
                nc.tensor.transpose(ps[:], T[ti][:], identf[0:NT, 0:NT])
                nc.scalar.copy(G[gi][:], ps[:])
                lay = "G"
        elif op[0] == "shuffle_rev":
            nc.vector.stream_shuffle(R[:], T[ti][:], mask=list(range(NT - 1, -1, -1)))
        else:
            _, slay, cxs = op
            assert slay == lay
            if lay == "G":
                cur, nxt = G[gi], G[1 - gi]
                rowsz, nparts = NT, NP
                gi = 1 - gi
            else:
                cur, nxt = T[ti], T[1 - ti]
                rowsz, nparts = NP, NT
                ti = 1 - ti
            for dst_sel, in0_sel, in1_sel, alu, in1_rev in cxs:
                src1 = R if in1_rev else cur
                nc.vector.tensor_tensor(
                    _sel_ap(nxt, dst_sel, rowsz, nparts),
                    _sel_ap(cur, in0_sel, rowsz, nparts),
                    _sel_ap(src1, in1_sel, rowsz, nparts),
                    op=OP.min if alu == "min" else OP.max,
                )
    assert lay == "G"
    return G[gi]


# ---------------------------------------------------------------------------
# Kernel program (SPMD: identical on all 8 cores; roles differ via inputs)
# ---------------------------------------------------------------------------

# K=21 class layout: (lhs block, rhs block) pairs, 3 rows each:
#  rows 0-2:  ones | yy h/m/l   rows 9-11: -2mA | hX   rows 18-20: -2mA | mX
#  rows 3-5:  -2hA | hX         rows 12-14:-2hA | lX
#  rows 6-8:  -2hA | mX         rows 15-17:-2lA | hX
# (ones rows at 0-2 so the engine memset starts at partition 0)
LHS_ROWS = {"h": (3, 6, 12), "m": (9, 18), "l": (15,)}
RHS_ROWS = {"h": (3, 9, 15), "m": (6, 18), "l": (12,)}


def _emit_program(nc, repeats=1):
    a_pts = nc.dram_tensor("a_pts", [N, 3], F32, kind="ExternalInput")
    b_pts = nc.dram_tensor("b_pts", [N, 3], F32, kind="ExternalInput")
    out_t = nc.dram_tensor("out", [1, 4], F32, kind="ExternalOutput")

    with tile.TileContext(nc) as tc:
        with (
            tc.tile_pool(name="const", bufs=1) as cst,
            tc.tile_pool(name="setup", bufs=1) as stp,
            tc.tile_pool(name="feat", bufs=1) as feat,
            tc.tile_pool(name="jobs", bufs=1) as jbs,
            tc.tile_pool(name="jpsum", bufs=1, space="PSUM") as jpsum,
            tc.tile_pool(name="tpsum", bufs=1, space="PSUM") as tpsum,
            tc.tile_pool(name="dram", bufs=1, space="DRAM") as dram,
        ):
          for _rep in range(repeats):
            sfx = f"_r{_rep}"
            # ---- constants
            identf = cst.tile([128, 128], F32)
            nc.vector.memset(identf[:], 0.0)
            nc.gpsimd.affine_select(
                identf[:], identf[:], pattern=[[-1, 128]],
                compare_op=OP.not_equal, fill=1.0, base=0, channel_multiplier=1,
            )
            identb = cst.tile([128, 128], BF16)
            nc.vector.memset(identb[:], 0.0)
            nc.gpsimd.affine_select(
                identb[:], identb[:], pattern=[[-1, 128]],
                compare_op=OP.not_equal, fill=1.0, base=0, channel_multiplier=1,
            )
            diagmask = cst.tile([128, 128], F32)
            nc.vector.memset(diagmask[:], 0.0)
            nc.gpsimd.affine_select(
                diagmask[:], diagmask[:], pattern=[[-1, 128]],
                compare_op=OP.not_equal, fill=DIAG_BIG, base=0, channel_multiplier=1,
            )

            FL = feat.tile([21, N], BF16)    # lhs features of A
            FRC = feat.tile([21, N], BF16)   # rhs features of B (cross)
            FRS = feat.tile([21, N], BF16)   # rhs features of A (self)
            nc.vector.memset(FL[0:3, :], 1.0)  # ones rows pair with yy splits

            def put3(stage_bf, F, rows):
                """stage_bf [96,128] (partition d*32+b, free p) -> F[r:r+3, :]
                for each r in rows, col enum j = b*128+p (flat reshape DMA)."""
                for r in rows:
                    nc.sync.dma_start(F[r : r + 3, :], stage_bf[:])

            def setup_set(pts, tag, make_lhs, F_rhs):
                """Load a point set, build split features.  Returns xx grid
                [128, 32] f32 (xx[p, t] = |point enum t*128+p|^2)."""
                gb = stp.tile([128, 96], F32, name=f"gb_{tag}{sfx}")
                nc.sync.dma_start(gb[:], pts[:].rearrange("(p b) d -> p (b d)", p=128))
                # d-major copy: gd[p, d*32+b] = gb[p, b*3+d]
                gd = stp.tile([128, 96], F32, name=f"gd_{tag}{sfx}")
                nc.vector.tensor_copy(
                    gd[:].rearrange("p (d b) -> p d b", d=3),
                    bass.AP(gb.tensor, gb.offset, [[96, 128], [1, 3], [3, 32]]),
                )
                # norms (b-major): xx[p, b] = sum_d gb[p, 3b+d]^2
                sq = stp.tile([128, 96], F32, name=f"sq_{tag}{sfx}")
                nc.scalar.activation(sq[:], gb[:], ACTF.Square)
                xxg = stp.tile([128, 32], F32, name=f"xx_{tag}{sfx}")
                nc.vector.tensor_reduce(
                    xxg[:], sq[:].rearrange("p (b d) -> p b d", d=3),
                    axis=AX.X, op=OP.add,
                )
                # exact 3-way bf16 split of coordinates (d-major grids)
                h = stp.tile([128, 96], BF16, name=f"h_{tag}{sfx}")
                nc.vector.tensor_copy(h[:], gd[:])
                r1 = stp.tile([128, 96], F32, name=f"r1_{tag}{sfx}")
                nc.vector.tensor_tensor(r1[:], gd[:], h[:], op=OP.subtract)
                mg = stp.tile([128, 96], BF16, name=f"m_{tag}{sfx}")
                nc.vector.tensor_copy(mg[:], r1[:])
                r2 = stp.tile([128, 96], F32, name=f"r2_{tag}{sfx}")
                nc.vector.tensor_tensor(r2[:], r1[:], mg[:], op=OP.subtract)
                lg = stp.tile([128, 96], BF16, name=f"l_{tag}{sfx}")
                nc.vector.tensor_copy(lg[:], r2[:])

                splits = {"h": h, "m": mg, "l": lg}
                # transpose each split [128,96] -> [96,128] and DMA into F rows
                for s, grid in splits.items():
                    ps = tpsum.tile([96, 128], BF16, tag="tp", bufs=2)
                    nc.tensor.transpose(ps[:], grid[:], identb[:])
                    st = stp.tile([96, 128], BF16, name=f"st_{s}_{tag}{sfx}")
                    nc.scalar.copy(st[:], ps[:])
                    put3(st, F_rhs, RHS_ROWS[s])
                    if make_lhs:
                        st2 = stp.tile([96, 128], BF16, name=f"st2_{s}_{tag}{sfx}")
                        nc.vector.tensor_scalar(st2[:], st[:], -2.0, None, OP.mult)
                        put3(st2, FL, LHS_ROWS[s])
                # yy rows: transpose xx grid -> [32, 128], 3-way split, rows 18-20
                yps = tpsum.tile([32, 128], F32, tag="tp", bufs=2)
                nc.tensor.transpose(yps[:], xxg[:], identf[:])
                yst = stp.tile([32, 128], F32, name=f"yst_{tag}{sfx}")
                nc.scalar.copy(yst[:], yps[:])
                yh = stp.tile([32, 128], BF16, name=f"yh_{tag}{sfx}")
                nc.vector.tensor_copy(yh[:], yst[:])
                yr1 = stp.tile([32, 128], F32, name=f"yr1_{tag}{sfx}")
                nc.vector.tensor_tensor(yr1[:], yst[:], yh[:], op=OP.subtract)
                ym = stp.tile([32, 128], BF16, name=f"ym_{tag}{sfx}")
                nc.vector.tensor_copy(ym[:], yr1[:])
                yr2 = stp.tile([32, 128], F32, name=f"yr2_{tag}{sfx}")
                nc.vector.tensor_tensor(yr2[:], yr1[:], ym[:], op=OP.subtract)
                yl = stp.tile([32, 128], BF16, name=f"yl_{tag}{sfx}")
                nc.vector.tensor_copy(yl[:], yr2[:])
                for i, yt in enumerate((yh, ym, yl)):
                    nc.sync.dma_start(F_rhs[i : i + 1, :], yt[:])
                return xxg

            xxA = setup_set(a_pts, "a", make_lhs=True, F_rhs=FRS)
            setup_set(b_pts, "b", make_lhs=False, F_rhs=FRC)

            # ---- distance jobs: rowmin over all 4096 cols per row (VectorE —
            # the only engine with a free-dim reduce).
            def job(F_rhs, diag, xxg, name):
                # level-1: one reduce per (t, c) chunk into M4 column t*4+c;
                # level-2: a single whole-job [128, 32, 4] -> [128, 32] reduce
                # plus one TT add of ||x||^2 (saves ~120 tiny DVE ops).
                M4 = jbs.tile([128, 128], F32, name=f"M4_{name}{sfx}")
                for t in range(32):
                    for c in range(4):
                        ps = jpsum.tile([128, 1024], F32, tag="jp", bufs=3)
                        lhsT = FL[:, t * 128 : (t + 1) * 128]
                        nc.tensor.matmul(
                            ps[:, 0:512], lhsT,
                            F_rhs[:, c * 1024 : c * 1024 + 512],
                            start=True, stop=True,
                        )
                        nc.tensor.matmul(
                            ps[:, 512:1024], lhsT,
                            F_rhs[:, c * 1024 + 512 : (c + 1) * 1024],
                            start=True, stop=True,
                        )
                        if diag and (t * 128) // 1024 == c:
                            off = (t * 128) % 1024
                            nc.vector.tensor_tensor(
                                ps[:, off : off + 128], ps[:, off : off + 128],
                                diagmask[:], op=OP.add,
                            )
                        col = t * 4 + c
                        nc.vector.tensor_reduce(
                            M4[:, col : col + 1], ps[:], axis=AX.X, op=OP.min
                        )
                M = jbs.tile([128, 32], F32, name=f"M_{name}{sfx}")
                nc.vector.tensor_reduce(
                    M[:], M4[:].rearrange("p (t c) -> p t c", c=4),
                    axis=AX.X, op=OP.min,
                )
                nc.vector.tensor_tensor(M[:], M[:], xxg[:], op=OP.add)
                return M

            Mself = job(FRS, True, xxA, "self")

            # ---- sum of squares of self mins
            msq = jbs.tile([128, 32], F32, name=f"msq{sfx}")
            nc.vector.tensor_tensor(msq[:], Mself[:], Mself[:], op=OP.mult)
            ssum = jbs.tile([128, 1], F32, name=f"ssum{sfx}")
            nc.vector.tensor_reduce(ssum[:], msq[:], axis=AX.X, op=OP.add)
            ssum_a = jbs.tile([128, 1], F32, name=f"ssum_a{sfx}")
            nc.gpsimd.partition_all_reduce(
                ssum_a[:], ssum[:], channels=128, reduce_op=bass_isa.ReduceOp.add
            )

            # ---- sort self mins (VectorE; overlaps the pool-lane cross job)
            SG = _emit_sort(nc, jbs, tpsum, Mself, identf, sfx)

            Mcross = job(FRC, False, xxA, "cross")

            # ---- partial scalar (sum of cross rowmins), all-partitions
            csum = jbs.tile([128, 1], F32, name=f"csum{sfx}")
            nc.vector.tensor_reduce(csum[:], Mcross[:], axis=AX.X, op=OP.add)
            csum_a = jbs.tile([128, 1], F32, name=f"csum_a{sfx}")
            nc.gpsimd.partition_all_reduce(
                csum_a[:], csum[:], channels=128, reduce_op=bass_isa.ReduceOp.add
            )

            # ---- payload: [sorted(4096), partial, ssum, pad...]
            pay = jbs.tile([1, 4104], F32, name=f"pay{sfx}")
            nc.sync.dma_start(
                pay[0:1, 0:4096].rearrange("o (p t) -> o p t", p=128), SG[:]
            )
            nc.vector.tensor_copy(pay[0:1, 4096:4097], csum_a[0:1, :])
            nc.vector.tensor_copy(pay[0:1, 4097:4098], ssum_a[0:1, :])
            nc.vector.memset(pay[0:1, 4098:4104], 0.0)
            cc_in = dram.tile([1, 4104], F32)
            cc_out = dram.tile([N_CORES, 4104], F32, addr_space="Shared")
            nc.sync.dma_start(cc_in[:], pay[:])
            nc.gpsimd.collective_compute(
                "AllGather", OP.bypass,
                replica_groups=[list(range(N_CORES))],
                ins=[cc_in[:]], outs=[cc_out[:]],
            )

            # ---- final combine (identical on every core)
            sgs = []
            for c in range(N_CORES):
                g = jbs.tile([128, 32], F32, name=f"fin_sg{c}{sfx}")
                nc.sync.dma_start(
                    g[:],
                    cc_out[c : c + 1, 0:4096].rearrange("o (p t) -> (o p) t", p=128),
                )
                sgs.append(g)
            scal = jbs.tile([8, 2], F32, name=f"fin_scal{sfx}")
            nc.sync.dma_start(scal[:], cc_out[:, 4096:4098])
            scrow = jbs.tile([1, 16], F32, name=f"fin_scrow{sfx}")
            nc.sync.dma_start(scrow[:], scal[:])
            drow = jbs.tile([1, 4], F32, name=f"fin_drow{sfx}")
            for b in range(4):
                pr = jbs.tile([128, 32], F32, tag="fin_pr", bufs=2)
                nc.vector.tensor_tensor(pr[:], sgs[2 * b][:], sgs[2 * b + 1][:], op=OP.mult)
                pc = jbs.tile([128, 1], F32, tag="fin_pc", bufs=2)
                nc.vector.tensor_reduce(pc[:], pr[:], axis=AX.X, op=OP.add)
                pa = jbs.tile([128, 1], F32, tag="fin_pa", bufs=2)
                nc.gpsimd.partition_all_reduce(
                    pa[:], pc[:], channels=128, reduce_op=bass_isa.ReduceOp.add
                )
                nc.vector.tensor_copy(drow[0:1, b : b + 1], pa[0:1, :])
            # out[b] = partial_2b + partial_2b+1 + ALPHA*(ss_2b + ss_2b+1 - 2*dot_b)
            t1 = jbs.tile([1, 4], F32, name=f"fin_t1{sfx}")
            nc.vector.tensor_tensor(
                t1[:],
                bass.AP(scrow.tensor, scrow.offset + 0, [[16, 1], [4, 4]]),
                bass.AP(scrow.tensor, scrow.offset + 2, [[16, 1], [4, 4]]),
                op=OP.add,
            )
            t2 = jbs.tile([1, 4], F32, name=f"fin_t2{sfx}")
            nc.vector.tensor_tensor(
                t2[:],
                bass.AP(scrow.tensor, scrow.offset + 1, [[16, 1], [4, 4]]),
                bass.AP(scrow.tensor, scrow.offset + 3, [[16, 1], [4, 4]]),
                op=OP.add,
            )
            t3 = jbs.tile([1, 4], F32, name=f"fin_t3{sfx}")
            # t3 = t1 + ALPHA * t2 ; ALPHA == 1.0
            nc.vector.tensor_tensor(t3[:], t1[:], t2[:], op=OP.add)
            res = jbs.tile([1, 4], F32, name=f"fin_res{sfx}")
            nc.vector.tensor_scalar(res[:], drow[:], -2.0 * ALPHA, None, OP.mult)
            nc.vector.tensor_tensor(res[:], res[:], t3[:], op=OP.add)
            nc.sync.dma_start(out_t[:], res[:])

    return nc


_CACHE = {}


def _get_nc(repeats=1):
    key = ("nc", repeats)
    if key not in _CACHE:
        nc = bacc.Bacc(
            "TRN2", target_bir_lowering=False, debug=False, num_devices=N_CORES
        )
        _emit_program(nc, repeats=repeats)
        nc.compile()
        _CACHE[key] = nc
    return _CACHE[key]


def make_in_maps(gts, preds):
    gts = np.ascontiguousarray(np.asarray(gts, dtype=np.float32))
    preds = np.ascontiguousarray(np.asarray(preds, dtype=np.float32))
    in_maps = []
    for c in range(N_CORES):
        b = c // 2
        if c % 2 == 0:
            a_set, b_set = gts[b], preds[b]
        else:
            a_set, b_set = preds[b], gts[b]
        in_maps.append(
            {"a_pts": np.ascontiguousarray(a_set), "b_pts": np.ascontiguousarray(b_set)}
        )
    return in_maps


def kernel(gts, preds):
    nc = _get_nc()
    in_maps = make_in_maps(gts, preds)
    res = run_bass_kernel_spmd(nc, in_maps, list(range(N_CORES)))
    return np.asarray(res.results[0]["out"][0], dtype=np.float32)

